# revision 1
# baseline (speedup 1.0000x reference)
"""Trainium2 Bass kernel for nn_BlockwiseEarlyExitMamba.

Strategy
--------
- Pure data parallel over batch B=128 across 8 NeuronCores (16 samples/core).
- Only timesteps t < 32 are computed: the model is causal (left-padded conv +
  forward scan) and the classifier reads position EXIT_POS-1 = 31, so
  timesteps 32..63 cannot affect the output.
- Feature-major layout on chip: channels on partitions, (state n, batch b,
  time t) along the free dimension.  The selective scan runs as a single
  `tensor_tensor_scan` per 128-channel tile with per-(n,b) segment resets.
- The causal depthwise conv is folded into the in_proj matmul on the host
  (K = 256 channels x 4 taps), reading the residual stream from a per-sample
  left-padded layout.
- The tokenizer (embedding gathers + projections + fusion matmul) is folded on
  the host into difference-weights so the whole thing becomes PE matmuls
  against `is_ge` step masks (Abel summation).
- B/C state projections are broadcast across partitions with a tiny
  DRAM round-trip DMA (partition-stride-0 read) instead of compute-engine ops.
- The decay factors exp(A_n * dt) are produced by per-n ScalarE Exp
  instructions with A_n baked as the activation scale immediate.
- Layer 4's readout / out-projection / LN run only at t=31.
"""

import sys

sys.path.insert(0, "/opt/trn_rl_repo")

import numpy as np

# model constants (hardcoded per problem spec)
D_MODEL = 256
D_INNER = 512
D_STATE = 16
D_CONV = 4
DT_RANK = 16
N_LAYERS = 4
B = 128
LFULL = 64
L = 32            # effective sequence length (early exit at 32)
NCORES = 8
BLOC = B // NCORES      # 16 samples per core
TOK = BLOC * L          # 512 tokens per core
LP = L + 3              # padded per-sample row (3 zeros for conv taps)
BLOBC = 8 * 512 + 2 * 512 + 4 * 256 + 4 * 48  # weight blob columns per layer
EPS = 1e-5
NDT = D_INNER // 128    # 4 channel tiles of 128
BIG = D_STATE * BLOC * L  # 8192 free elements per scan tile

_cache: dict = {}


def _build_program(A_vals, sim_compat=False):
    """Build + compile the SPMD Bass program.

    A_vals: [N_LAYERS][D_STATE] python floats; baked as Exp scale immediates.
    sim_compat=True replaces Silu (not implemented in CoreSim) with
    sigmoid*x; the hardware program uses native Silu.
    """
    import concourse.bass as bass
    import concourse.bacc as bacc
    import concourse.tile as tile
    from concourse import mybir

    f32, f16 = mybir.dt.float32, mybir.dt.float16
    AF = mybir.ActivationFunctionType
    OP = mybir.AluOpType

    nc = bacc.Bacc("TRN2", num_devices=NCORES)

    # ---------------- DRAM tensors ----------------
    d_in = {}

    def din(name, shape, dt):
        d_in[name] = nc.dram_tensor(name, list(shape), dt, kind="ExternalInput").ap()
        return d_in[name]

    xrows = din("xrows", [5, TOK], f32)          # proto,len,flags,iat,dir rows
    din("iota0", [128, 1], f32)
    din("iota1", [128, 1], f32)
    din("Wp", [256, D_MODEL], f16)               # diff-folded proto weights
    din("Wf", [64, D_MODEL], f16)
    din("wlen", [1, D_MODEL], f16)
    din("wiat", [1, D_MODEL], f16)
    din("wdir", [1, D_MODEL], f16)
    din("bfused", [1, D_MODEL], f16)
    din("tokg_row", [1, D_MODEL], f16)
    din("tokb_row", [1, D_MODEL], f16)
    din("tokg_col", [128, 2], f32)
    din("lng_row", [1, D_MODEL], f16)
    din("lnb_row", [1, D_MODEL], f16)
    din("lng_col", [128, 2], f32)
    din("selid", [16, 16], f16)
    din("ones128", [128, 1], f16)
    din("w1T", [D_MODEL, 128], f16)
    din("b1", [128, 1], f32)
    din("w2T", [128, 2], f16)
    din("b2", [2, 1], f32)
    # per-layer weight blob, SBUF layout [128, BLOBC]:
    #   iucw 8*512 | ipwz 2*512 | opwT 4*256 | xpwT 4*48
    for l in range(N_LAYERS):
        din(f"wblob{l}", [128, BLOBC], f16)
        din(f"dpwT{l}", [DT_RANK, D_INNER], f16)
        din(f"cols{l}", [D_INNER, 3], f32)       # convb, dpb, dskip packed

    logits_d = nc.dram_tensor("logits", [2, BLOC], f32, kind="ExternalOutput").ap()
    bcscr = [nc.dram_tensor(f"bcscr{l}", [32, TOK], f16, kind="Internal").ap()
             for l in range(N_LAYERS)]

    with tile.TileContext(nc) as tc:
        ctx_pools = []

        def mkpool(**kw):
            p = tc.tile_pool(**kw)
            pool = p.__enter__()
            ctx_pools.append(p)
            return pool

        wp = mkpool(name="weights", bufs=1)
        wrot = mkpool(name="wrot", bufs=2)       # rotating per-layer big weights
        sp = mkpool(name="small", bufs=1)
        featp = mkpool(name="feat", bufs=2)
        psp = mkpool(name="ps", bufs=8, space="PSUM")
        tokp_cm = tc.tile_pool(name="tok", bufs=1)
        tokp = tokp_cm.__enter__()

        def ps_tile(shape):
            return psp.tile(list(shape), f32, tag="ps", name="ps")

        from concourse.tile import add_dep_helper
        act_chain = [None]

        import os
        _chain_on = os.environ.get("ACT_CHAIN", "1") == "1"

        def act(**kw):
            inst = nc.scalar.activation(**kw)
            if _chain_on:
                if act_chain[0] is not None:
                    add_dep_helper(inst.ins, act_chain[0], sync=False,
                                   reason="ACT table-set grouping")
                act_chain[0] = inst.ins
            return inst

        W = {}

        def wtile(name, shape, dt, src_ap, pool=None, q=None, tag=None):
            t = (pool or wp).tile(list(shape), dt, tag=tag or name, name=name)
            (q or nc.sync).dma_start(out=t, in_=src_ap)
            W[name] = t
            return t

        def tmp(shape, dt, tag, bufs=2):
            return sp.tile(list(shape), dt, tag=tag, bufs=bufs, name=tag)

        def act_silu(out, in_, bias=0.0):
            if not sim_compat:
                act(out=out, in_=in_, func=AF.Silu, bias=bias)
            else:
                n2 = out.shape[-1] if len(out.shape) == 2 else TOK
                sg = sp.tile([128, TOK], f16, tag="silu_sg", bufs=1,
                             name="silu_sg")[:, 0:n2]
                idn = sp.tile([128, TOK], f16, tag="silu_id", bufs=1,
                              name="silu_id")[:, 0:n2]
                act(out=sg, in_=in_, func=AF.Sigmoid, bias=bias)
                act(out=idn, in_=in_, func=AF.Identity, bias=bias)
                nc.vector.tensor_mul(out, sg, idn)

        # ------------- tokenizer-only loads -------------
        xr_proto = wtile("xr_proto", [1, TOK], f32, xrows[0:1, :], tokp)
        xr_len = wtile("xr_len", [1, TOK], f32, xrows[1:2, :], tokp)
        xr_flags = wtile("xr_flags", [1, TOK], f32, xrows[2:3, :], tokp)
        xr_iat = wtile("xr_iat", [1, TOK], f32, xrows[3:4, :], tokp)
        xr_dir = wtile("xr_dir", [1, TOK], f32, xrows[4:5, :], tokp)
        io0 = wtile("iota0", [128, 1], f32, d_in["iota0"], tokp)
        io1 = wtile("iota1", [128, 1], f32, d_in["iota1"], tokp)
        Wp0 = wtile("Wp0", [128, D_MODEL], f16, d_in["Wp"][0:128, :], tokp)
        Wp1 = wtile("Wp1", [128, D_MODEL], f16, d_in["Wp"][128:256, :], tokp)
        Wf = wtile("Wf", [64, D_MODEL], f16, d_in["Wf"], tokp)
        wlen = wtile("wlen", [1, D_MODEL], f16, d_in["wlen"], tokp)
        wiat = wtile("wiat", [1, D_MODEL], f16, d_in["wiat"], tokp)
        wdir = wtile("wdir", [1, D_MODEL], f16, d_in["wdir"], tokp)
        bfu = wtile("bfused", [1, D_MODEL], f16, d_in["bfused"], tokp)
        tokg_row = wtile("tokg_row", [1, D_MODEL], f16, d_in["tokg_row"], tokp)
        tokb_row = wtile("tokb_row", [1, D_MODEL], f16, d_in["tokb_row"], tokp)
        tokg_col = wtile("tokg_col", [128, 2], f32, d_in["tokg_col"], tokp)
        ones1x128_f32 = tokp.tile([1, 128], f32, tag="ones1x128f32",
                                  name="ones1x128f32")
        nc.vector.memset(ones1x128_f32, 1.0)

        # ------------- persistent weights -------------
        lng_row = wtile("lng_row", [1, D_MODEL], f16, d_in["lng_row"])
        lnb_row = wtile("lnb_row", [1, D_MODEL], f16, d_in["lnb_row"])
        lng_col = wtile("lng_col", [128, 2], f32, d_in["lng_col"])
        selid = wtile("selid", [16, 16], f16, d_in["selid"])
        ones128 = wtile("ones128", [128, 1], f16, d_in["ones128"])
        w1T0 = wtile("w1T0", [128, 128], f16, d_in["w1T"][0:128, :])
        w1T1 = wtile("w1T1", [128, 128], f16, d_in["w1T"][128:256, :])
        b1c = wtile("b1", [128, 1], f32, d_in["b1"])
        w2T = wtile("w2T", [128, 2], f16, d_in["w2T"])
        b2c = wtile("b2", [2, 1], f32, d_in["b2"])
        for l in range(N_LAYERS):
            wtile(f"dpwT{l}", [16, D_INNER], f16, d_in[f"dpwT{l}"])
            # packed per-channel columns: [128, k, (convb,dpb,dskip)]
            src = d_in[f"cols{l}"].rearrange("(k p) j -> p k j", k=NDT)
            wtile(f"cols{l}", [128, NDT, 3], f32, src)

        onesrow = wp.tile([1, TOK], f16, tag="onesrow", name="onesrow")
        nc.vector.memset(onesrow, 1.0)
        ones128_f32 = wp.tile([128, 1], f32, tag="ones128f32", name="ones128f32")
        nc.vector.memset(ones128_f32, 1.0)
        epsc = wp.tile([128, 1], f32, tag="epsc", name="epsc")
        nc.vector.memset(epsc, EPS)

        # =======================================================
        # LayerNorm over the channel axis (256 ch = 2 partition tiles)
        # out[m] = (x*rstd)*g + (b - mu*rstd*g)  via PE-broadcast helpers
        # =======================================================
        def layer_norm(n_tok, rs, g_row, b_row, g_col, out_tiles):
            for m in range(2):
                xsq = tmp([128, n_tok], f32, "ln_xsq", bufs=1)
                act(out=xsq, in_=rs[m], func=AF.Square)
                if m == 0:
                    ps_s = ps_tile([1, TOK])
                    ps_s2 = ps_tile([1, TOK])
                nc.tensor.matmul(out=ps_s[:, 0:n_tok], lhsT=ones128[:, 0:1],
                                 rhs=rs[m], start=(m == 0), stop=(m == 1))
                nc.tensor.matmul(out=ps_s2[:, 0:n_tok], lhsT=ones128_f32,
                                 rhs=xsq, start=(m == 0), stop=(m == 1))
            mu = tmp([1, TOK], f32, "ln_mu", bufs=1)[:, 0:n_tok]
            tA = tmp([1, TOK], f32, "ln_tA", bufs=1)[:, 0:n_tok]
            tB = tmp([1, TOK], f32, "ln_tB", bufs=1)[:, 0:n_tok]
            nc.vector.tensor_scalar_mul(mu, ps_s[:, 0:n_tok], 1.0 / D_MODEL)
            nc.vector.tensor_scalar_mul(tA, ps_s2[:, 0:n_tok], 1.0 / D_MODEL)
            nc.vector.tensor_mul(tB, mu, mu)
            nc.vector.tensor_sub(tA, tA, tB)        # var = m2 - mu^2 (in place)
            rstd16 = tmp([1, TOK], f16, "ln_rstd16", bufs=1)[:, 0:n_tok]
            # rstd = exp(-0.5*ln(var+eps)) (ln/exp stay in the exp-family sets)
            act(out=tA, in_=tA, func=AF.Ln, bias=epsc[0:1, :])
            act(out=rstd16, in_=tA, func=AF.Exp, scale=-0.5)
            nc.vector.tensor_mul(tB, mu, rstd16)
            nmur16 = tmp([1, TOK], f16, "ln_nmur16", bufs=1)[:, 0:n_tok]
            act(out=nmur16, in_=tB, func=AF.Copy, scale=-1.0)
            ps_R = ps_tile([128, TOK])[:, 0:n_tok]
            nc.tensor.matmul(out=ps_R, lhsT=ones128[0:1, 0:1].broadcast_to([1, 128]),
                             rhs=rstd16, start=True, stop=True)
            for m in range(2):
                ps_D = ps_tile([128, TOK])[:, 0:n_tok]
                nc.tensor.matmul(out=ps_D, lhsT=g_row[0:1, m * 128:(m + 1) * 128],
                                 rhs=nmur16, start=True, stop=False)
                nc.tensor.matmul(out=ps_D, lhsT=b_row[0:1, m * 128:(m + 1) * 128],
                                 rhs=onesrow[:, 0:n_tok], start=False, stop=True)
                t1 = tmp([128, n_tok], f16, "ln_t1", bufs=1)
                nc.vector.tensor_mul(t1, rs[m], ps_R)
                nc.vector.scalar_tensor_tensor(
                    out=out_tiles[m], in0=t1, scalar=g_col[:, m:m + 1], in1=ps_D,
                    op0=OP.mult, op1=OP.add)

        # =======================================================
        # Tokenizer
        # =======================================================
        ps_pb = ps_tile([128, TOK])
        nc.tensor.matmul(out=ps_pb, lhsT=ones1x128_f32, rhs=xr_proto,
                         start=True, stop=True)
        ps_fb = ps_tile([128, TOK])
        nc.tensor.matmul(out=ps_fb, lhsT=ones1x128_f32, rhs=xr_flags,
                         start=True, stop=True)
        ge_p0 = tokp.tile([128, TOK], f16, tag="ge_p0", name="ge_p0")
        ge_p1 = tokp.tile([128, TOK], f16, tag="ge_p1", name="ge_p1")
        ge_f = tokp.tile([64, TOK], f16, tag="ge_f", name="ge_f")
        nc.vector.tensor_scalar(out=ge_p0, in0=ps_pb, scalar1=io0, scalar2=None,
                                op0=OP.is_ge)
        nc.vector.tensor_scalar(out=ge_p1, in0=ps_pb, scalar1=io1, scalar2=None,
                                op0=OP.is_ge)
        nc.vector.tensor_scalar(out=ge_f, in0=ps_fb[0:64, :], scalar1=io0[0:64],
                                scalar2=None, op0=OP.is_ge)
        dir01 = tokp.tile([1, TOK], f16, tag="dir01", name="dir01")
        nc.vector.tensor_scalar(out=dir01, in0=xr_dir, scalar1=1.0,
                                scalar2=None, op0=OP.is_ge)
        len16 = tokp.tile([1, TOK], f16, tag="len16", name="len16")
        iat16 = tokp.tile([1, TOK], f16, tag="iat16", name="iat16")
        act(out=len16, in_=xr_len, func=AF.Copy)
        act(out=iat16, in_=xr_iat, func=AF.Copy)

        tok_rs = [tmp([128, TOK], f16, f"rs{m}", bufs=1) for m in range(2)]
        for m in range(2):
            ps_tok = ps_tile([128, TOK])
            ms = slice(m * 128, (m + 1) * 128)
            nc.tensor.matmul(out=ps_tok, lhsT=Wp0[:, ms], rhs=ge_p0, start=True, stop=False)
            nc.tensor.matmul(out=ps_tok, lhsT=Wp1[:, ms], rhs=ge_p1, start=False, stop=False)
            nc.tensor.matmul(out=ps_tok, lhsT=Wf[:, ms], rhs=ge_f, start=False, stop=False)
            nc.tensor.matmul(out=ps_tok, lhsT=wlen[:, ms], rhs=len16, start=False, stop=False)
            nc.tensor.matmul(out=ps_tok, lhsT=wiat[:, ms], rhs=iat16, start=False, stop=False)
            nc.tensor.matmul(out=ps_tok, lhsT=wdir[:, ms], rhs=dir01, start=False, stop=False)
            nc.tensor.matmul(out=ps_tok, lhsT=bfu[:, ms], rhs=onesrow, start=False, stop=True)
            act(out=tok_rs[m], in_=ps_tok, func=AF.Copy)

        # residual stream: per-sample left-padded rows [128, 2 * BLOC * LP]
        def new_feat():
            f = featp.tile([128, 2 * BLOC * LP], f16, tag="feat", name="feat")
            f4 = f.rearrange("p (m b c) -> p m b c", m=2, b=BLOC)
            nc.gpsimd.memset(f4[:, :, :, 0:3], 0.0)
            return f, f4

        feat, feat4 = new_feat()
        layer_norm(TOK, tok_rs, tokg_row, tokb_row, tokg_col,
                   [feat4[:, 0, :, 3:LP], feat4[:, 1, :, 3:LP]])

        tokp_cm.__exit__(None, None, None)
        bigp = mkpool(name="big", bufs=1)
        bcp = mkpool(name="bc", bufs=1)

        # =======================================================
        # Mamba layers
        # =======================================================
        feat31 = None
        for l in range(N_LAYERS):
            last = (l == N_LAYERS - 1)
            blob = wtile(f"wblob{l}", [128, BLOBC], f16, d_in[f"wblob{l}"],
                         wrot, tag="wblob")
            iucw = [blob[:, j * 512:(j + 1) * 512] for j in range(8)]
            ipwz = [blob[:, 4096 + k * 512:4096 + (k + 1) * 512] for k in range(2)]
            opwT = [blob[:, 5120 + k * 256:5120 + (k + 1) * 256] for k in range(NDT)]
            xpwT = [blob[:, 6144 + k * 48:6144 + (k + 1) * 48] for k in range(NDT)]
            cols = W[f"cols{l}"]

            # ---- in_proj(u) + folded conv;  in_proj(z) + silu ----
            uc = [tmp([128, TOK], f16, f"uc{k}", bufs=1) for k in range(NDT)]
            siluz = [tmp([128, TOK], f16, f"sz{k}", bufs=1) for k in range(NDT)]
            for m in range(NDT):
                ps_u = ps_tile([128, TOK])
                ms = slice(m * 128, (m + 1) * 128)
                j = 0
                for kk in range(2):
                    for tap in range(D_CONV):
                        nc.tensor.matmul(
                            out=ps_u, lhsT=iucw[kk * D_CONV + tap][:, ms],
                            rhs=feat4[:, kk, :, tap:tap + L],
                            start=(j == 0), stop=(j == 2 * D_CONV - 1))
                        j += 1
                act_silu(uc[m], ps_u, bias=cols[:, m, 0:1])
            def emit_z_block():
                for m in range(NDT):
                    ps_z = ps_tile([128, TOK])
                    ms = slice(m * 128, (m + 1) * 128)
                    for kk in range(2):
                        nc.tensor.matmul(out=ps_z, lhsT=ipwz[kk][:, ms],
                                         rhs=feat4[:, kk, :, 3:LP],
                                         start=(kk == 0), stop=(kk == 1))
                    act_silu(siluz[m], ps_z)

            # ---- xproj (split into dt/B/C; each lands at partition base 0) ----
            ps_dtl = ps_tile([16, TOK])
            ps_Bm = ps_tile([16, TOK])
            ps_Cm = ps_tile([16, TOK])
            for k in range(NDT):
                nc.tensor.matmul(out=ps_dtl, lhsT=xpwT[k][:, 0:16],
                                 rhs=uc[k], start=(k == 0), stop=(k == NDT - 1))
                nc.tensor.matmul(out=ps_Bm, lhsT=xpwT[k][:, 16:32],
                                 rhs=uc[k], start=(k == 0), stop=(k == NDT - 1))
                nc.tensor.matmul(out=ps_Cm, lhsT=xpwT[k][:, 32:48],
                                 rhs=uc[k], start=(k == 0), stop=(k == NDT - 1))
            dtlow = tmp([16, TOK], f16, "dtlow", bufs=1)
            act(out=dtlow, in_=ps_dtl[0:16, :], func=AF.Copy)
            Bm = tmp([16, TOK], f16, "Bm", bufs=1)
            act(out=Bm, in_=ps_Bm[0:16, :], func=AF.Copy)
            Cm = tmp([16, TOK], f16, "Cm", bufs=1)
            act(out=Cm, in_=ps_Cm[0:16, :], func=AF.Copy)

            # ---- B_rep / C_rep via DRAM round-trip partition broadcast ----
            nc.sync.dma_start(out=bcscr[l][0:16, :], in_=Bm)
            B_rep = bcp.tile([128, BIG], f16, tag="B_rep", name="B_rep")
            for hh in range(2):
                nc.sync.dma_start(
                    out=B_rep[:, hh * (BIG // 2):(hh + 1) * (BIG // 2)],
                    in_=bcscr[l][8 * hh:8 * (hh + 1), :].rearrange("n t -> (n t)")
                    .rearrange("(o f) -> o f", o=1).broadcast_to([128, BIG // 2]))
            if not last:
                nc.sync.dma_start(out=bcscr[l][16:32, :], in_=Cm)
                C_rep = bcp.tile([128, BIG], f16, tag="C_rep", name="C_rep")
                for hh in range(2):
                    nc.sync.dma_start(
                        out=C_rep[:, hh * (BIG // 2):(hh + 1) * (BIG // 2)],
                        in_=bcscr[l][16 + 8 * hh:16 + 8 * (hh + 1), :]
                        .rearrange("n t -> (n t)")
                        .rearrange("(o f) -> o f", o=1).broadcast_to([128, BIG // 2]))
            else:
                C31 = tmp([128, D_STATE * BLOC], f16, "C31")
                for n in range(D_STATE):
                    ps_c = ps_tile([128, TOK])
                    nc.tensor.matmul(
                        out=ps_c[:, 0:BLOC],
                        lhsT=selid[:, n:n + 1].broadcast_to([16, 128]),
                        rhs=Cm.rearrange("q (b t) -> q b t", b=BLOC)[:, :, L - 1],
                        start=True, stop=True)
                    nc.vector.tensor_copy(C31[:, n * BLOC:(n + 1) * BLOC],
                                          ps_c[:, 0:BLOC])

            # ---- dt projection + softplus + dt*u ----
            dtv = [tmp([128, TOK], f16, f"dtv{k}", bufs=1) for k in range(NDT)]
            dtu = [tmp([128, TOK], f16, f"dtu{k}", bufs=1) for k in range(NDT)]
            for k in range(NDT):
                ps_dt = ps_tile([128, TOK])
                nc.tensor.matmul(out=ps_dt,
                                 lhsT=W[f"dpwT{l}"][:, k * 128:(k + 1) * 128],
                                 rhs=dtlow, start=True, stop=True)
                act(out=dtv[k], in_=ps_dt, func=AF.Exp, bias=cols[:, k, 1:2])
            for k in range(NDT):
                act(out=dtv[k], in_=dtv[k], func=AF.Ln, bias=1.0)
            for k in range(NDT):
                nc.vector.tensor_mul(dtu[k], dtv[k], uc[k])

            # ---- per channel-tile: decay, scan input, scan, readout ----
            y16 = [None] * NDT
            y31 = [None] * NDT
            HB = BIG // 2
            NH = D_STATE // 2
            for k in range(NDT):
                a16 = bigp.tile([128, BIG], f16, tag="a", bufs=2, name="a16")
                a4 = a16.rearrange("p (n b t) -> p n b t", n=D_STATE, b=BLOC)
                x16 = bigp.tile([128, BIG], f16, tag="x", bufs=1, name="x16")
                h16 = bigp.tile([128, BIG], f16, tag="h", bufs=1, name="h16")
                hc = (bigp.tile([128, BIG], f16, tag="hc", bufs=1, name="hc")
                      if not last else None)
                dtu_b = (dtu[k].rearrange("p (b t) -> p b t", b=BLOC)
                         .unsqueeze(1).broadcast_to([128, NH, BLOC, L]))
                for hh in range(2):
                    hs = slice(hh * HB, (hh + 1) * HB)
                    for n in range(NH * hh, NH * (hh + 1)):
                        act(out=a16[:, n * TOK:(n + 1) * TOK],
                            in_=dtv[k], func=AF.Exp, scale=float(A_vals[l][n]))
                    nc.gpsimd.memset(a4[:, NH * hh:NH * (hh + 1), :, 0:1], 0.0)
                    x4h = (x16[:, hs].rearrange("p (n b t) -> p n b t",
                                                n=NH, b=BLOC))
                    nc.vector.tensor_tensor(out=x4h, in0=dtu_b,
                                            in1=B_rep[:, hs], op=OP.mult)
                    nc.vector.tensor_tensor_scan(
                        out=h16[:, hs], data0=a16[:, hs], data1=x16[:, hs],
                        initial=0.0, op0=OP.mult, op1=OP.add)
                    if not last:
                        nc.vector.tensor_mul(hc[:, hs], h16[:, hs], C_rep[:, hs])
                        half = HB // 2
                        while half >= TOK:
                            nc.vector.tensor_add(
                                hc[:, hh * HB:hh * HB + half],
                                hc[:, hh * HB:hh * HB + half],
                                hc[:, hh * HB + half:hh * HB + 2 * half])
                            half //= 2
                if not last:
                    nc.vector.tensor_add(hc[:, 0:TOK], hc[:, 0:TOK],
                                         hc[:, HB:HB + TOK])
                    y16[k] = hc  # y = hc[:, 0:TOK]
                if last:
                    h31 = h16.rearrange("p (n b t) -> p n b t", n=D_STATE,
                                        b=BLOC)[:, :, :, L - 1:L]
                    hc31 = tmp([128, D_STATE * BLOC], f16, "hc31")
                    nc.vector.tensor_tensor(
                        out=hc31.rearrange("p (n b) -> p n b", n=D_STATE)
                        .unsqueeze(3),
                        in0=h31,
                        in1=C31.rearrange("p (n b) -> p n b", n=D_STATE)
                        .unsqueeze(3),
                        op=OP.mult)
                    half = D_STATE * BLOC // 2
                    while half >= BLOC:
                        nc.vector.tensor_add(hc31[:, 0:half], hc31[:, 0:half],
                                             hc31[:, half:2 * half])
                        half //= 2
                    y31[k] = hc31  # y at t=31: hc31[:, 0:BLOC]

            # ---- skip, gate, out_proj, residual + LN ----
            emit_z_block()
            if not last:
                y3 = [tmp([128, TOK], f16, f"y3_{k}", bufs=1) for k in range(NDT)]
                for k in range(NDT):
                    y2 = tmp([128, TOK], f16, "y2", bufs=1)
                    nc.vector.scalar_tensor_tensor(
                        out=y2, in0=uc[k], scalar=cols[:, k, 2:3],
                        in1=y16[k][:, 0:TOK], op0=OP.mult, op1=OP.add)
                    nc.vector.tensor_mul(y3[k], y2, siluz[k])
                rs = [tmp([128, TOK], f16, f"rs{m}", bufs=1) for m in range(2)]
                for m in range(2):
                    ps_o = ps_tile([128, TOK])
                    ms = slice(m * 128, (m + 1) * 128)
                    for k in range(NDT):
                        nc.tensor.matmul(out=ps_o, lhsT=opwT[k][:, ms],
                                         rhs=y3[k], start=(k == 0), stop=(k == NDT - 1))
                    nc.vector.tensor_add(rs[m], ps_o, feat4[:, m, :, 3:LP])
                feat, feat4 = new_feat()
                layer_norm(TOK, rs, lng_row, lnb_row, lng_col,
                           [feat4[:, 0, :, 3:LP], feat4[:, 1, :, 3:LP]])
            else:
                nt = BLOC
                y3 = [tmp([128, TOK], f16, f"y3_{k}", bufs=1)[:, 0:nt]
                      for k in range(NDT)]
                for k in range(NDT):
                    uc31 = (uc[k].rearrange("p (b t) -> p b t", b=BLOC)
                            [:, :, L - 1:L]).squeeze(2)
                    sz31 = (siluz[k].rearrange("p (b t) -> p b t", b=BLOC)
                            [:, :, L - 1:L]).squeeze(2)
                    y2 = tmp([128, TOK], f16, "y2", bufs=1)[:, 0:nt]
                    nc.vector.scalar_tensor_tensor(
                        out=y2, in0=uc31, scalar=cols[:, k, 2:3],
                        in1=y31[k][:, 0:nt], op0=OP.mult, op1=OP.add)
                    nc.vector.tensor_mul(y3[k], y2, sz31)
                rs = [tmp([128, TOK], f16, f"rs{m}", bufs=1)[:, 0:nt] for m in range(2)]
                for m in range(2):
                    ps_o = ps_tile([128, TOK])
                    ms = slice(m * 128, (m + 1) * 128)
                    for k in range(NDT):
                        nc.tensor.matmul(out=ps_o[:, 0:nt], lhsT=opwT[k][:, ms],
                                         rhs=y3[k], start=(k == 0), stop=(k == NDT - 1))
                    f31 = feat4[:, m, :, LP - 1:LP].squeeze(2)
                    nc.vector.tensor_add(rs[m], ps_o[:, 0:nt], f31)
                feat31 = [tmp([128, BLOC], f16, f"feat31_{m}") for m in range(2)]
                layer_norm(nt, rs, lng_row, lnb_row, lng_col, feat31)

        # =======================================================
        # Classifier head
        # =======================================================
        ps_h1 = ps_tile([128, TOK])
        nc.tensor.matmul(out=ps_h1[:, 0:BLOC], lhsT=w1T0, rhs=feat31[0],
                         start=True, stop=False)
        nc.tensor.matmul(out=ps_h1[:, 0:BLOC], lhsT=w1T1, rhs=feat31[1],
                         start=False, stop=True)
        r1 = tmp([128, BLOC], f16, "r1")
        act(out=r1, in_=ps_h1[:, 0:BLOC], func=AF.Relu, bias=b1c)
        ps_lg = ps_tile([2, TOK])
        nc.tensor.matmul(out=ps_lg[:, 0:BLOC], lhsT=w2T, rhs=r1, start=True, stop=True)
        lg = tmp([2, BLOC], f32, "lg")
        act(out=lg, in_=ps_lg[:, 0:BLOC], func=AF.Identity, bias=b2c)
        nc.sync.dma_start(out=logits_d, in_=lg)

        for p in reversed(ctx_pools):
            p.__exit__(None, None, None)

    nc.compile()
    return nc


def _fold_weights(inp):
    """Host-side weight preprocessing (weights only, no activation data)."""
    f16 = np.float16
    fw = np.asarray(inp["fusion_w"], np.float32)          # [256, 136]
    emb_proto = np.asarray(inp["emb_proto"], np.float32)  # [256, 32]
    emb_flags = np.asarray(inp["emb_flags"], np.float32)  # [64, 32]
    emb_dir = np.asarray(inp["emb_dir"], np.float32)      # [2, 8]
    plw = np.asarray(inp["proj_len_w"], np.float32)       # [32, 1]
    plb = np.asarray(inp["proj_len_b"], np.float32)
    piw = np.asarray(inp["proj_iat_w"], np.float32)
    pib = np.asarray(inp["proj_iat_b"], np.float32)

    Wp = emb_proto @ fw[:, 0:32].T                        # [256, 256]
    Wp = np.vstack([Wp[0:1], np.diff(Wp, axis=0)])
    Wf = emb_flags @ fw[:, 64:96].T                       # [64, 256]
    Wf = np.vstack([Wf[0:1], np.diff(Wf, axis=0)])
    wlen = (fw[:, 32:64] @ plw).T                         # [1, 256]
    wiat = (fw[:, 96:128] @ piw).T
    wdir = (fw[:, 128:136] @ (emb_dir[1] - emb_dir[0]))[None, :]
    bfused = (np.asarray(inp["fusion_b"], np.float32)
              + fw[:, 32:64] @ plb + fw[:, 96:128] @ pib
              + fw[:, 128:136] @ emb_dir[0])[None, :]

    w = {
        "iota0": np.arange(128, dtype=np.float32).reshape(128, 1),
        "iota1": np.arange(128, 256, dtype=np.float32).reshape(128, 1),
        "Wp": Wp.astype(f16), "Wf": Wf.astype(f16),
        "wlen": wlen.astype(f16), "wiat": wiat.astype(f16),
        "wdir": np.ascontiguousarray(wdir).astype(f16),
        "bfused": bfused.astype(f16),
        "tokg_row": np.asarray(inp["tok_norm_g"], np.float32)[None, :].astype(f16),
        "tokb_row": np.asarray(inp["tok_norm_b"], np.float32)[None, :].astype(f16),
        "tokg_col": np.asarray(inp["tok_norm_g"], np.float32).reshape(2, 128).T.copy(),
        "lng_row": np.asarray(inp["norm_g"], np.float32)[None, :].astype(f16),
        "lnb_row": np.asarray(inp["norm_b"], np.float32)[None, :].astype(f16),
        "lng_col": np.asarray(inp["norm_g"], np.float32).reshape(2, 128).T.copy(),
        "selid": np.eye(16, dtype=f16),
        "ones128": np.ones((128, 1), dtype=f16),
        "w1T": np.ascontiguousarray(np.asarray(inp["cls_w1"], np.float32).T).astype(f16),
        "b1": np.asarray(inp["cls_b1"], np.float32).reshape(128, 1),
        "w2T": np.ascontiguousarray(np.asarray(inp["cls_w2"], np.float32).T).astype(f16),
        "b2": np.asarray(inp["cls_b2"], np.float32).reshape(2, 1),
    }
    A_vals = []
    for l in range(N_LAYERS):
        ipw = np.asarray(inp["in_proj_w"][l], np.float32)    # [1024, 256]
        convw = np.asarray(inp["conv_w"][l], np.float32)     # [512, 4]
        ipw_u = ipw[0:D_INNER, :]                            # [512, 256]
        # iucw[(kk, tap, c), d] = ipw_u[d, kk*128+c] * convw[d, tap]
        blocks = []
        for kk in range(2):
            for tap in range(D_CONV):
                blocks.append((ipw_u[:, kk * 128:(kk + 1) * 128]
                               * convw[:, tap:tap + 1]).T)   # [128, 512]
        iucw = np.concatenate(blocks, 0)                     # [1024, 512]
        ipwzT = ipw[D_INNER:, :].T                           # [256, 512]
        opwT = np.asarray(inp["outproj_w"][l], np.float32).T  # [512, 256]
        xpwT = np.asarray(inp["xproj_w"][l], np.float32).T    # [512, 48]
        parts = []
        for j in range(8):
            parts.append(iucw[j * 128:(j + 1) * 128, :])      # [128, 512]
        for k in range(2):
            parts.append(ipwzT[k * 128:(k + 1) * 128, :])     # [128, 512]
        for k in range(4):
            parts.append(opwT[k * 128:(k + 1) * 128, :])      # [128, 256]
        for k in range(4):
            parts.append(xpwT[k * 128:(k + 1) * 128, :])      # [128, 48]
        w[f"wblob{l}"] = np.ascontiguousarray(
            np.concatenate(parts, axis=1)).astype(f16)
        w[f"dpwT{l}"] = np.ascontiguousarray(
            np.asarray(inp["dtproj_w"][l], np.float32).T).astype(f16)
        w[f"cols{l}"] = np.ascontiguousarray(np.stack(
            [np.asarray(inp["conv_b"][l], np.float32),
             np.asarray(inp["dtproj_b"][l], np.float32),
             np.asarray(inp["D_skip"][l], np.float32)], axis=1))
        A_vals.append([float(a) for a in
                       -np.exp(np.asarray(inp["A_log"], np.float32)[l, 0])])
    return w, A_vals


def _make_in_maps(inp):
    w, A_vals = _fold_weights(inp)
    x = np.asarray(inp["x"], np.float32)
    in_maps = []
    for c in range(NCORES):
        m = dict(w)
        xs = x[c * BLOC:(c + 1) * BLOC, :L, :]            # [16, 32, 5]
        m["xrows"] = np.ascontiguousarray(xs.transpose(2, 0, 1).reshape(5, TOK))
        in_maps.append(m)
    return in_maps, A_vals


def _get_nc(A_vals, sim_compat=False):
    key = ("nc_sim" if sim_compat else "nc", tuple(np.round(
        np.asarray(A_vals, np.float64).ravel(), 9)))
    if key not in _cache:
        _cache[key] = _build_program(A_vals, sim_compat=sim_compat)
    return _cache[key]


def kernel(**inputs):
    from concourse.bass_utils import run_bass_kernel_spmd

    in_maps, A_vals = _make_in_maps(inputs)
    nc = _get_nc(A_vals)
    res = run_bass_kernel_spmd(nc, in_maps, core_ids=list(range(NCORES)))
    out = np.zeros((B, 2), dtype=np.float32)
    for c in range(NCORES):
        out[c * BLOC:(c + 1) * BLOC, :] = res.results[c]["logits"].T
    return out


def run_core_sim(inputs, core=0):
    """Debug helper: run one core in CoreSim and return logits for that shard."""
    from concourse.bass_interp import CoreSim

    in_maps, A_vals = _make_in_maps(inputs)
    nc = _get_nc(A_vals, sim_compat=True)
    sim = CoreSim(nc, trace=False)
    for name, arr in in_maps[core].items():
        sim.tensor(name)[:] = arr
    sim.simulate(check_with_hw=False)
    return sim.tensor("logits").T.copy()

